# revision 1
# baseline (speedup 1.0000x reference)
"""CSPN (7x7 per-pixel spatial propagation) Trainium2 kernel.

Problem: out[b,0,y,x] = sum_{i,j in 0..6} gw[b, 7i+j, y+3, x+3] * src(y+3-i, x+3-j)
where src = hn (zero-padded outside [0,512)) except the center tap (i=j=3)
which uses h0. Shapes: gw [8,49,518,518] f32, hn/h0 [8,1,512,512] f32.

Strategy: pure data parallel - one batch element per NeuronCore (8 cores).
Per core the 512x512 image lives as [128 partitions, 4 row-blocks, 512
cols]. The guide-weight read window is identical for every tap (rows/cols
3:515), so each tap is one ~1MB DMA; that 51.4MB/core stream is the
memory-roofline term.

Engine/queue roles (chosen to avoid sequencer convoys - a HWDGE dma_start
waiting on a semaphore blocks every later instruction on that sequencer):
 - Sync + GpSimd sequencers: pure DMA issue rings for the weight stream
   (round-robin), so the 16 SDMA engines always have a second descriptor
   ring to drain during one ring's completion gap.
 - Scalar engine: only the f32->bf16 weight casts (so DVE multiplies run
   in 2x mode) plus the final output stores.
 - Vector engine: per-tap multiply + 49-term bf16 accumulation, halo
   plane casts.

The per-tap source shift is absorbed by a zero-padded bf16 halo tensor
s0[p, k, b, u] = hn[128b+p+k-3, u-3]; a second copy s1 one u-slot later
keeps bf16 reads 4B-aligned for odd-j taps. Each partition-shifted plane
is staged in f32 by SBUF->SBUF DMA from the raw hn staging tile (no cast
dependency - engine ops cannot partition-shift, DMAs can) on the GpSimd
ring, then cast to s0/s1 on the DVE. Planes build one image-row ahead of
the tap chain. The last three taps run block-striped (per-row-block
weight quarters, accumulate, cast, store) so the tail drains as a
pipeline behind the final weight bytes.
"""

import numpy as np

_CACHE = {}


def _build_nc():
    import concourse.bacc as bacc
    import concourse.mybir as mybir
    import concourse.tile as tile

    F32 = mybir.dt.float32
    BF16 = mybir.dt.bfloat16
    MULT = mybir.AluOpType.mult
    ADD = mybir.AluOpType.add

    nc = bacc.Bacc("TRN2", target_bir_lowering=False, debug=False, num_devices=8)
    gw = nc.dram_tensor("gw", [49, 518, 518], F32, kind="ExternalInput").ap()
    hn = nc.dram_tensor("hn", [512, 512], F32, kind="ExternalInput").ap()
    h0 = nc.dram_tensor("h0", [512, 512], F32, kind="ExternalInput").ap()
    out = nc.dram_tensor("out", [512, 512], F32, kind="ExternalOutput").ap()

    with tile.TileContext(nc) as tc:
        with (
            tc.tile_pool(name="persist", bufs=1) as pp,
            tc.tile_pool(name="wf", bufs=7) as wfp,
            tc.tile_pool(name="ftmp", bufs=2) as ftp,
            tc.tile_pool(name="wb", bufs=5) as wbp,
            tc.tile_pool(name="prod", bufs=3) as prp,
            tc.tile_pool(name="wtail", bufs=5) as wtp,
        ):
            # Stage hn/h0 as [p, b, x]; h0's bf16 cast runs on Scalar.
            hn_r = hn.rearrange("(b p) x -> p b x", p=128)
            hnf = pp.tile([128, 4, 512], F32, tag="stage_a")
            nc.sync.dma_start(out=hnf[:], in_=hn_r)
            h0f = pp.tile([128, 4, 512], F32)
            nc.sync.dma_start(out=h0f[:], in_=h0.rearrange("(b p) x -> p b x", p=128))
            h0b = pp.tile([128, 4, 512], BF16)
            nc.scalar.copy(out=h0b[:], in_=h0f[:])

            # Halo tensors: s0[p, k, b, u] = hn[128b+p+k-3, u-3] (zero outside
            # the image), s1 the same data one u-slot later so odd-j taps read
            # 4B-aligned.
            s0 = pp.tile([128, 7, 4, 520], BF16, tag="s0")
            s1 = pp.tile([128, 7, 4, 520], BF16, tag="s1")
            nc.vector.memset(s0[:, :, :, 0:3], 0.0)
            nc.vector.memset(s0[:, :, :, 515:520], 0.0)
            nc.vector.memset(s1[:, :, :, 0:4], 0.0)
            nc.vector.memset(s1[:, :, :, 516:520], 0.0)
            # Zero strip used to clear halo staging gap rows via DMA (DMAs
            # have no partition-alignment constraints, engine memsets do).
            zt = pp.tile([32, 512], F32, tag="zt")
            nc.vector.memset(zt[:], 0.0)

            def build_plane(k):
                d = k - 3
                if d == 0:
                    # Unshifted plane: cast straight from hnf on the DVE.
                    nc.vector.tensor_copy(s0[:, 3, :, 3:515], hnf[:])
                    nc.vector.tensor_copy(s1[:, 3, :, 4:516], hnf[:])
                    return
                # Partition-shifted plane staged in f32 straight from DRAM hn
                # (no dependencies, normal HBM->SBUF direction - SBUF->SBUF
                # staging starves against the weight stream's port traffic),
                # then cast to both bf16 copies on the DVE. Gap rows hold
                # garbage in the staging tile; they are re-zeroed in s0/s1
                # right after the casts (same DVE FIFO, no ring stalls).
                ft = ftp.tile([128, 4, 512], F32, tag="ftmp")
                eng = nc.sync if k % 2 == 0 else nc.scalar
                if d > 0:
                    eng.dma_start(out=ft[0 : 128 - d, 0:4, :], in_=hn_r[d:128, 0:4, :])
                    eng.dma_start(out=ft[128 - d : 128, 0:3, :], in_=hn_r[0:d, 1:4, :])
                    eng.dma_start(out=ft[128 - d : 128, 3, :], in_=zt[0:d, :])
                else:
                    eng.dma_start(out=ft[-d:128, 0:4, :], in_=hn_r[0 : 128 + d, 0:4, :])
                    eng.dma_start(out=ft[0:-d, 1:4, :], in_=hn_r[128 + d : 128, 0:3, :])
                    eng.dma_start(out=ft[0:-d, 0, :], in_=zt[0:-d, :])
                nc.vector.tensor_copy(s0[:, k, :, 3:515], ft[:])
                nc.vector.tensor_copy(s1[:, k, :, 4:516], ft[:])

            acc = pp.tile([128, 4, 512], BF16)
            outf = pp.tile([128, 4, 512], F32, tag="stage_a")
            out_ap = out.rearrange("(b p) x -> p b x", p=128)

            def src_for(t):
                i, j = t // 7, t % 7
                if t == 24:
                    return h0b[:]
                if j % 2 == 0:
                    return s0[:, 6 - i, :, 6 - j : 518 - j]
                return s1[:, 6 - i, :, 7 - j : 519 - j]

            # Taps 0..45 stream full-tile on the two pure-DMA rings; plane
            # k=6-i is built one image-row ahead of the taps that read it.
            build_plane(6)
            for t in range(46):
                i, j = t // 7, t % 7
                if j == 0 and i < 6:
                    build_plane(5 - i)
                wf = wfp.tile([128, 4, 512], F32, tag="wf")
                eng = nc.sync if t % 2 == 0 else nc.scalar
                eng.dma_start(
                    out=wf[:],
                    in_=gw[t, 3:515, 3:515].rearrange("(b p) x -> p b x", p=128),
                )
                # bf16 weight cast on the Scalar engine (2x DVE multiply).
                wb = wbp.tile([128, 4, 512], BF16, tag="wb")
                nc.scalar.copy(out=wb[:], in_=wf[:])
                if t == 0:
                    nc.vector.tensor_tensor(
                        out=acc[:], in0=wb[:], in1=src_for(t), op=MULT
                    )
                else:
                    prod = prp.tile([128, 4, 512], BF16, tag="prod")
                    nc.vector.tensor_tensor(
                        out=prod[:], in0=wb[:], in1=src_for(t), op=MULT
                    )
                    nc.vector.tensor_tensor(
                        out=acc[:], in0=acc[:], in1=prod[:], op=ADD
                    )

            # Tail: last three taps run block-striped (block 0's weight
            # quarters first) so each block's accumulate/cast/store drains
            # while later blocks' weights are still arriving.
            for b in range(4):
                for t in (46, 47, 48):
                    wq = wtp.tile([128, 512], F32, tag="wq")
                    eng = nc.sync if t % 2 == 0 else nc.scalar
                    eng.dma_start(
                        out=wq[:], in_=gw[t, 3 + 128 * b : 131 + 128 * b, 3:515]
                    )
                    wbq = wtp.tile([128, 512], BF16, tag="wbq")
                    nc.scalar.copy(out=wbq[:], in_=wq[:])
                    prod = prp.tile([128, 512], BF16, tag="prodb")
                    nc.vector.tensor_tensor(
                        out=prod[:], in0=wbq[:], in1=src_for(t)[:, b, :], op=MULT
                    )
                    nc.vector.tensor_tensor(
                        out=acc[:, b, :], in0=acc[:, b, :], in1=prod[:], op=ADD
                    )
                nc.scalar.copy(out=outf[:, b, :], in_=acc[:, b, :])
                nc.scalar.dma_start(out=out_ap[:, b, :], in_=outf[:, b, :])

    nc.compile()
    return nc


def get_nc():
    if "nc" not in _CACHE:
        _CACHE["nc"] = _build_nc()
    return _CACHE["nc"]


def kernel(guide_weight, hn, h0):
    from concourse.bass_utils import run_bass_kernel_spmd

    nc = get_nc()
    in_maps = [
        {
            "gw": np.ascontiguousarray(guide_weight[b], dtype=np.float32),
            "hn": np.ascontiguousarray(hn[b, 0], dtype=np.float32),
            "h0": np.ascontiguousarray(h0[b, 0], dtype=np.float32),
        }
        for b in range(8)
    ]
    res = run_bass_kernel_spmd(nc, in_maps, core_ids=list(range(8)))
    return np.stack([res.results[b]["out"] for b in range(8)])[:, None].astype(
        np.float32
    )



# revision 3
# speedup vs baseline: 1.1065x; 1.1065x over previous
"""CSPN (7x7 per-pixel spatial propagation) Trainium2 kernel.

Problem: out[b,0,y,x] = sum_{i,j in 0..6} gw[b, 7i+j, y+3, x+3] * src(y+3-i, x+3-j)
where src = hn (zero-padded outside [0,512)) except the center tap (i=j=3)
which uses h0. Shapes: gw [8,49,518,518] f32, hn/h0 [8,1,512,512] f32.

Strategy: pure data parallel - one batch element per NeuronCore (8 cores).
Per core the 512x512 image lives as [128 partitions, 4 row-blocks, 512
cols]. The guide-weight read window is identical for every tap (rows/cols
3:515), so each tap is one ~1MB DMA; that 51.4MB/core stream is the
memory-roofline term. Everything else is kept OFF the DMA engines:

 - The 6 row-shifted halo planes are produced on the (otherwise idle)
   TensorEngine as shifted-identity matmuls into PSUM (main diagonal for
   the intra-partition shift, a second carry-diagonal matmul accumulating
   the <=3 rows that cross the 128-partition block boundary), then cast
   PSUM->bf16 into the halo tensors by the DVE. The old approach re-read
   hn from HBM once per plane (~6MB extra DMA).
 - Shift matrices are built in SBUF with affine_select (no DMA).
 - Only the final tap (t=48) is block-striped into 4 quarter DMAs so the
   per-block accumulate/cast/store drains behind the last weight bytes;
   all other taps are full-tile, keeping the weight stream free of
   compute-gated DMA gaps.

The per-tap column shift is absorbed by the zero-padded bf16 halo tensor
s0[p, k, b, u] = hn[128b+p+k-3, u-3]; a second copy s1 one u-slot later
keeps bf16 reads 4B-aligned for odd-j taps. Weight f32->bf16 casts run on
the Scalar engine (2x DVE multiply rate); per-tap multiply + accumulate
on the Vector engine; tail output casts on GpSimd; stores on the Sync
ring.
"""

import numpy as np

_CACHE = {}


def _build_nc():
    import concourse.bacc as bacc
    import concourse.mybir as mybir
    import concourse.tile as tile

    F32 = mybir.dt.float32
    BF16 = mybir.dt.bfloat16
    MULT = mybir.AluOpType.mult
    ADD = mybir.AluOpType.add
    EQ = mybir.AluOpType.is_equal

    nc = bacc.Bacc("TRN2", target_bir_lowering=False, debug=False, num_devices=8)
    gw = nc.dram_tensor("gw", [49, 518, 518], F32, kind="ExternalInput").ap()
    hn = nc.dram_tensor("hn", [512, 512], F32, kind="ExternalInput").ap()
    h0 = nc.dram_tensor("h0", [512, 512], F32, kind="ExternalInput").ap()
    out = nc.dram_tensor("out", [512, 512], F32, kind="ExternalOutput").ap()

    with tile.TileContext(nc) as tc:
        with (
            tc.tile_pool(name="persist", bufs=1) as pp,
            tc.tile_pool(name="wf", bufs=7) as wfp,
            tc.tile_pool(name="wb", bufs=5) as wbp,
            tc.tile_pool(name="prod", bufs=3) as prp,
            tc.tile_pool(name="wtail", bufs=4) as wtp,
            tc.tile_pool(name="psum", bufs=2, space="PSUM") as psp,
        ):
            hn_r = hn.rearrange("(b p) x -> p b x", p=128)
            hnf = pp.tile([128, 4, 512], F32)
            nc.sync.dma_start(out=hnf[:], in_=hn_r)
            h0f = pp.tile([128, 4, 512], F32)
            nc.scalar.dma_start(out=h0f[:], in_=h0.rearrange("(b p) x -> p b x", p=128))
            hnb = pp.tile([128, 4, 512], BF16)
            nc.scalar.copy(out=hnb[:], in_=hnf[:])
            h0b = pp.tile([128, 4, 512], BF16)
            nc.scalar.copy(out=h0b[:], in_=h0f[:])

            # Shift matrices for the TensorEngine halo-plane builds.
            # Wm[k, c] = 1 iff c == k+3; the lhsT view Wm[:, 3+d:131+d] is then
            # [k, m] = 1 iff k == m+d, i.e. out[m] = hn_r[m+d] (rows that stay
            # within the partition block; out-of-range rows come out 0, which
            # is exactly the zero padding beyond the image edge).
            # Wc holds the two carry diagonals (c == k+131 for d>0, c == k-125
            # for d<0); the same view indexing turns it into the <=3-row
            # cross-block carry matrix, accumulated from block b+/-1.
            Wm = pp.tile([128, 134], BF16)
            Wc = pp.tile([128, 134], BF16)
            nc.vector.memset(Wm[:], 1.0)
            nc.vector.memset(Wc[:], 1.0)
            nc.gpsimd.affine_select(
                out=Wm[:], in_=Wm[:], pattern=[[1, 134]], compare_op=EQ,
                fill=0.0, base=-3, channel_multiplier=-1,
            )
            nc.gpsimd.affine_select(
                out=Wc[:, 0:3], in_=Wc[:, 0:3], pattern=[[1, 3]], compare_op=EQ,
                fill=0.0, base=125, channel_multiplier=-1,
            )
            nc.gpsimd.affine_select(
                out=Wc[:, 3:134], in_=Wc[:, 3:134], pattern=[[1, 131]], compare_op=EQ,
                fill=0.0, base=-128, channel_multiplier=-1,
            )

            # Halo tensors: s0[p, k, b, u] = hn[128b+p+k-3, u-3] (zero outside
            # the image), s1 the same data one u-slot later so odd-j taps read
            # 4B-aligned.
            s0 = pp.tile([128, 7, 4, 520], BF16, tag="s0")
            s1 = pp.tile([128, 7, 4, 520], BF16, tag="s1")
            nc.vector.memset(s0[:, :, :, 0:3], 0.0)
            nc.vector.memset(s0[:, :, :, 515:520], 0.0)
            nc.vector.memset(s1[:, :, :, 0:4], 0.0)
            nc.vector.memset(s1[:, :, :, 516:520], 0.0)

            def build_plane(k):
                d = k - 3
                if d == 0:
                    nc.vector.tensor_copy(s0[:, 3, :, 3:515], hnb[:])
                    nc.vector.tensor_copy(s1[:, 3, :, 4:516], hnb[:])
                    return
                sgn = 1 if d > 0 else -1
                pt = psp.tile([128, 4, 512], F32, tag="pt")
                for b in range(4):
                    carry = 0 <= b + sgn <= 3
                    nc.tensor.matmul(
                        pt[:, b, :], Wm[:, 3 + d : 131 + d], hnb[:, b, :],
                        start=True, stop=not carry,
                    )
                    if carry:
                        nc.tensor.matmul(
                            pt[:, b, :], Wc[:, 3 + d : 131 + d], hnb[:, b + sgn, :],
                            start=False, stop=True,
                        )
                nc.vector.tensor_copy(s0[:, k, :, 3:515], pt[:])
                nc.vector.tensor_copy(s1[:, k, :, 4:516], pt[:])

            acc = pp.tile([128, 4, 512], BF16)
            outf = pp.tile([128, 4, 512], F32)
            out_ap = out.rearrange("(b p) x -> p b x", p=128)

            def src_for(t):
                i, j = t // 7, t % 7
                if t == 24:
                    return h0b[:]
                if j % 2 == 0:
                    return s0[:, 6 - i, :, 6 - j : 518 - j]
                return s1[:, 6 - i, :, 7 - j : 519 - j]

            # Taps 0..47 stream full-tile on the two pure-DMA rings; plane
            # k=6-i is built one image-row ahead of the taps that read it.
            build_plane(6)
            for t in range(48):
                i, j = t // 7, t % 7
                if j == 0 and i < 6:
                    build_plane(5 - i)
                wf = wfp.tile([128, 4, 512], F32, tag="wf")
                eng = nc.sync if t % 2 == 0 else nc.scalar
                eng.dma_start(
                    out=wf[:],
                    in_=gw[t, 3:515, 3:515].rearrange("(b p) x -> p b x", p=128),
                )
                # bf16 weight cast on the Scalar engine (2x DVE multiply).
                wb = wbp.tile([128, 4, 512], BF16, tag="wb")
                nc.scalar.copy(out=wb[:], in_=wf[:])
                if t == 0:
                    nc.vector.tensor_tensor(
                        out=acc[:], in0=wb[:], in1=src_for(t), op=MULT
                    )
                else:
                    prod = prp.tile([128, 4, 512], BF16, tag="prod")
                    nc.vector.tensor_tensor(
                        out=prod[:], in0=wb[:], in1=src_for(t), op=MULT
                    )
                    nc.vector.tensor_tensor(
                        out=acc[:], in0=acc[:], in1=prod[:], op=ADD
                    )

            # Tail: the last tap runs block-striped so each block's
            # accumulate/cast/store drains while the remaining weight
            # quarters are still arriving. All 4 quarter DMAs are issued
            # before any compute-gated instruction lands on those rings.
            wqs = []
            for b in range(4):
                wq = wtp.tile([128, 512], F32, tag="wq")
                eng = nc.sync if b % 2 == 0 else nc.scalar
                eng.dma_start(
                    out=wq[:], in_=gw[48, 3 + 128 * b : 131 + 128 * b, 3:515]
                )
                wqs.append(wq)
            s48 = src_for(48)
            for b in range(4):
                wbq = wtp.tile([128, 512], BF16, tag="wbq")
                nc.scalar.copy(out=wbq[:], in_=wqs[b][:])
                prod = prp.tile([128, 512], BF16, tag="prodb")
                nc.vector.tensor_tensor(
                    out=prod[:], in0=wbq[:], in1=s48[:, b, :], op=MULT
                )
                nc.vector.tensor_tensor(
                    out=acc[:, b, :], in0=acc[:, b, :], in1=prod[:], op=ADD
                )
                nc.gpsimd.tensor_copy(out=outf[:, b, :], in_=acc[:, b, :])
                nc.sync.dma_start(out=out_ap[:, b, :], in_=outf[:, b, :])

    nc.compile()
    return nc


def get_nc():
    if "nc" not in _CACHE:
        _CACHE["nc"] = _build_nc()
    return _CACHE["nc"]


def kernel(guide_weight, hn, h0):
    from concourse.bass_utils import run_bass_kernel_spmd

    nc = get_nc()
    in_maps = [
        {
            "gw": np.ascontiguousarray(guide_weight[b], dtype=np.float32),
            "hn": np.ascontiguousarray(hn[b, 0], dtype=np.float32),
            "h0": np.ascontiguousarray(h0[b, 0], dtype=np.float32),
        }
        for b in range(8)
    ]
    res = run_bass_kernel_spmd(nc, in_maps, core_ids=list(range(8)))
    return np.stack([res.results[b]["out"] for b in range(8)])[:, None].astype(
        np.float32
    )


# revision 5
# speedup vs baseline: 1.1435x; 1.0335x over previous
"""CSPN (7x7 per-pixel spatial propagation) Trainium2 kernel.

Problem: out[b,0,y,x] = sum_{i,j in 0..6} gw[b, 7i+j, y+3, x+3] * src(y+3-i, x+3-j)
where src = hn (zero-padded outside [0,512)) except the center tap (i=j=3)
which uses h0. Shapes: gw [8,49,518,518] f32, hn/h0 [8,1,512,512] f32.

Strategy: pure data parallel - one batch element per NeuronCore (8 cores).
Per core the 512x512 image lives as [128 partitions, 4 row-blocks, 512
cols]. The guide-weight read window is identical for every tap (rows/cols
3:515), so each tap is one ~1MB DMA; that 51.4MB/core stream is the
memory-roofline term. Everything else is kept OFF the DMA engines:

 - The 6 row-shifted halo planes are produced on the (otherwise idle)
   TensorEngine as shifted-identity matmuls into PSUM (main diagonal for
   the intra-partition shift, a second carry-diagonal matmul accumulating
   the <=3 rows that cross the 128-partition block boundary), then cast
   PSUM->bf16 into the halo tensors by the DVE. The old approach re-read
   hn from HBM once per plane (~6MB extra DMA).
 - Shift matrices are built in SBUF with affine_select (no DMA).
 - Only the final tap (t=48) is block-striped into 4 quarter DMAs so the
   per-block accumulate/cast/store drains behind the last weight bytes;
   all other taps are full-tile, keeping the weight stream free of
   compute-gated DMA gaps.

The per-tap column shift is absorbed by the zero-padded bf16 halo tensor
s0[p, k, b, u] = hn[128b+p+k-3, u-3]; a second copy s1 one u-slot later
keeps bf16 reads 4B-aligned for odd-j taps. Weight f32->bf16 casts run on
the Scalar engine (2x DVE multiply rate); per-tap multiply + accumulate
on the Vector engine; tail output casts on GpSimd; stores on the Sync
ring.
"""

import numpy as np

_CACHE = {}


def _build_nc():
    import concourse.bacc as bacc
    import concourse.mybir as mybir
    import concourse.tile as tile

    F32 = mybir.dt.float32
    BF16 = mybir.dt.bfloat16
    MULT = mybir.AluOpType.mult
    ADD = mybir.AluOpType.add
    EQ = mybir.AluOpType.is_equal

    nc = bacc.Bacc("TRN2", target_bir_lowering=False, debug=False, num_devices=8)
    gw = nc.dram_tensor("gw", [49, 518, 518], F32, kind="ExternalInput").ap()
    hn = nc.dram_tensor("hn", [512, 512], F32, kind="ExternalInput").ap()
    h0 = nc.dram_tensor("h0", [512, 512], F32, kind="ExternalInput").ap()
    out = nc.dram_tensor("out", [512, 512], F32, kind="ExternalOutput").ap()

    with tile.TileContext(nc) as tc:
        with (
            tc.tile_pool(name="persist", bufs=1) as pp,
            tc.tile_pool(name="wf", bufs=7) as wfp,
            tc.tile_pool(name="wb", bufs=5) as wbp,
            tc.tile_pool(name="prod", bufs=3) as prp,
            tc.tile_pool(name="wtail", bufs=4) as wtp,
            tc.tile_pool(name="psum", bufs=2, space="PSUM") as psp,
        ):
            hn_r = hn.rearrange("(b p) x -> p b x", p=128)
            hnf = pp.tile([128, 4, 512], F32)
            nc.sync.dma_start(out=hnf[:], in_=hn_r)
            h0f = pp.tile([128, 4, 512], F32)
            nc.scalar.dma_start(out=h0f[:], in_=h0.rearrange("(b p) x -> p b x", p=128))
            hnb = pp.tile([128, 4, 512], BF16)
            nc.scalar.copy(out=hnb[:], in_=hnf[:])
            h0b = pp.tile([128, 4, 512], BF16)
            nc.scalar.copy(out=h0b[:], in_=h0f[:])

            # Shift matrices for the TensorEngine halo-plane builds.
            # Wm[k, c] = 1 iff c == k+3; the lhsT view Wm[:, 3+d:131+d] is then
            # [k, m] = 1 iff k == m+d, i.e. out[m] = hn_r[m+d] (rows that stay
            # within the partition block; out-of-range rows come out 0, which
            # is exactly the zero padding beyond the image edge).
            # Wc holds the two carry diagonals (c == k+131 for d>0, c == k-125
            # for d<0); the same view indexing turns it into the <=3-row
            # cross-block carry matrix, accumulated from block b+/-1.
            Wm = pp.tile([128, 134], BF16)
            Wc = pp.tile([128, 134], BF16)
            nc.vector.memset(Wm[:], 1.0)
            nc.vector.memset(Wc[:], 1.0)
            nc.gpsimd.affine_select(
                out=Wm[:], in_=Wm[:], pattern=[[1, 134]], compare_op=EQ,
                fill=0.0, base=-3, channel_multiplier=-1,
            )
            nc.gpsimd.affine_select(
                out=Wc[:, 0:3], in_=Wc[:, 0:3], pattern=[[1, 3]], compare_op=EQ,
                fill=0.0, base=125, channel_multiplier=-1,
            )
            nc.gpsimd.affine_select(
                out=Wc[:, 3:134], in_=Wc[:, 3:134], pattern=[[1, 131]], compare_op=EQ,
                fill=0.0, base=-128, channel_multiplier=-1,
            )

            # Halo tensors: s0[p, k, b, u] = hn[128b+p+k-3, u-3] (zero outside
            # the image), s1 the same data one u-slot later so odd-j taps read
            # 4B-aligned.
            s0 = pp.tile([128, 7, 4, 520], BF16, tag="s0")
            s1 = pp.tile([128, 7, 4, 520], BF16, tag="s1")
            nc.vector.memset(s0[:, :, :, 0:3], 0.0)
            nc.vector.memset(s0[:, :, :, 515:520], 0.0)
            nc.vector.memset(s1[:, :, :, 0:4], 0.0)
            nc.vector.memset(s1[:, :, :, 516:520], 0.0)

            def build_plane(k):
                d = k - 3
                if d == 0:
                    nc.vector.tensor_copy(s0[:, 3, :, 3:515], hnb[:])
                    nc.vector.tensor_copy(s1[:, 3, :, 4:516], hnb[:])
                    return
                sgn = 1 if d > 0 else -1
                pt = psp.tile([128, 4, 512], F32, tag="pt")
                for b in range(4):
                    carry = 0 <= b + sgn <= 3
                    nc.tensor.matmul(
                        pt[:, b, :], Wm[:, 3 + d : 131 + d], hnb[:, b, :],
                        start=True, stop=not carry,
                    )
                    if carry:
                        nc.tensor.matmul(
                            pt[:, b, :], Wc[:, 3 + d : 131 + d], hnb[:, b + sgn, :],
                            start=False, stop=True,
                        )
                nc.vector.tensor_copy(s0[:, k, :, 3:515], pt[:])
                nc.vector.tensor_copy(s1[:, k, :, 4:516], pt[:])

            acc = pp.tile([128, 4, 512], BF16)
            outf = pp.tile([128, 4, 512], F32)
            out_ap = out.rearrange("(b p) x -> p b x", p=128)

            def src_for(t):
                i, j = t // 7, t % 7
                if t == 24:
                    return h0b[:]
                if j % 2 == 0:
                    return s0[:, 6 - i, :, 6 - j : 518 - j]
                return s1[:, 6 - i, :, 7 - j : 519 - j]

            # Taps 0..47 stream full-tile, all issued from the Sync ring:
            # that ring carries no compute-gated instruction, so DMA issue
            # never convoys behind a cast's semaphore wait. Plane k=6-i is
            # built one image-row ahead of the taps that read it.
            build_plane(6)
            for t in range(48):
                i, j = t // 7, t % 7
                if j == 0 and i < 6:
                    build_plane(5 - i)
                wf = wfp.tile([128, 4, 512], F32, tag="wf")
                nc.sync.dma_start(
                    out=wf[:],
                    in_=gw[t, 3:515, 3:515].rearrange("(b p) x -> p b x", p=128),
                )
                # bf16 weight cast on the Scalar engine (2x DVE multiply).
                wb = wbp.tile([128, 4, 512], BF16, tag="wb")
                nc.scalar.copy(out=wb[:], in_=wf[:])
                if t == 0:
                    nc.vector.tensor_tensor(
                        out=acc[:], in0=wb[:], in1=src_for(t), op=MULT
                    )
                else:
                    prod = prp.tile([128, 4, 512], BF16, tag="prod")
                    nc.vector.tensor_tensor(
                        out=prod[:], in0=wb[:], in1=src_for(t), op=MULT
                    )
                    nc.vector.tensor_tensor(
                        out=acc[:], in0=acc[:], in1=prod[:], op=ADD
                    )

            # Tail: the last tap runs block-striped, in f32, fused into the
            # output: outf[:,b] = acc[:,b] + wq_b * s48_b. No bf16 cast and
            # no separate output cast sit on the critical drain; the per-
            # block products are even computed before acc's final add lands.
            # Blocks split DVE/GpSimd so the four drains overlap.
            wqs = []
            for b in range(4):
                wq = wtp.tile([128, 512], F32, tag="wq")
                nc.sync.dma_start(
                    out=wq[:], in_=gw[48, 3 + 128 * b : 131 + 128 * b, 3:515]
                )
                wqs.append(wq)
            s48 = src_for(48)
            prodfs = []
            for b in range(4):
                eng = nc.vector if b < 2 else nc.gpsimd
                prodf = wtp.tile([128, 512], F32, tag="prodf")
                eng.tensor_tensor(
                    out=prodf[:], in0=wqs[b][:], in1=s48[:, b, :], op=MULT
                )
                prodfs.append(prodf)
            for b in range(4):
                eng = nc.vector if b < 2 else nc.gpsimd
                eng.tensor_tensor(
                    out=outf[:, b, :], in0=acc[:, b, :], in1=prodfs[b][:], op=ADD
                )
                nc.sync.dma_start(out=out_ap[:, b, :], in_=outf[:, b, :])

    nc.compile()
    return nc


def get_nc():
    if "nc" not in _CACHE:
        _CACHE["nc"] = _build_nc()
    return _CACHE["nc"]


def kernel(guide_weight, hn, h0):
    from concourse.bass_utils import run_bass_kernel_spmd

    nc = get_nc()
    in_maps = [
        {
            "gw": np.ascontiguousarray(guide_weight[b], dtype=np.float32),
            "hn": np.ascontiguousarray(hn[b, 0], dtype=np.float32),
            "h0": np.ascontiguousarray(h0[b, 0], dtype=np.float32),
        }
        for b in range(8)
    ]
    res = run_bass_kernel_spmd(nc, in_maps, core_ids=list(range(8)))
    return np.stack([res.results[b]["out"] for b in range(8)])[:, None].astype(
        np.float32
    )


# revision 7
# speedup vs baseline: 1.1472x; 1.0032x over previous
"""CSPN (7x7 per-pixel spatial propagation) Trainium2 kernel.

Problem: out[b,0,y,x] = sum_{i,j in 0..6} gw[b, 7i+j, y+3, x+3] * src(y+3-i, x+3-j)
where src = hn (zero-padded outside [0,512)) except the center tap (i=j=3)
which uses h0. Shapes: gw [8,49,518,518] f32, hn/h0 [8,1,512,512] f32.

Strategy: pure data parallel - one batch element per NeuronCore (8 cores).
Per core the 512x512 image lives as [128 partitions, 4 row-blocks, 512
cols]. The guide-weight read window is identical for every tap (rows/cols
3:515), so each tap is one ~1MB DMA; that 51.4MB/core stream is the
memory-roofline term. Everything else is kept OFF the DMA engines:

 - The 6 row-shifted halo planes are produced on the (otherwise idle)
   TensorEngine as shifted-identity matmuls into PSUM (main diagonal for
   the intra-partition shift, a second carry-diagonal matmul accumulating
   the <=3 rows that cross the 128-partition block boundary), then cast
   PSUM->bf16 into the halo tensors by the DVE. The old approach re-read
   hn from HBM once per plane (~6MB extra DMA).
 - Shift matrices are built in SBUF with affine_select (no DMA).
 - Only the final tap (t=48) is block-striped into 4 quarter DMAs so the
   per-block accumulate/cast/store drains behind the last weight bytes;
   all other taps are full-tile, keeping the weight stream free of
   compute-gated DMA gaps.

The per-tap column shift is absorbed by the zero-padded bf16 halo tensor
s0[p, k, b, u] = hn[128b+p+k-3, u-3]; a second copy s1 one u-slot later
keeps bf16 reads 4B-aligned for odd-j taps. Weight f32->bf16 casts run on
the Scalar engine (2x DVE multiply rate); per-tap multiply + accumulate
on the Vector engine; tail output casts on GpSimd; stores on the Sync
ring.
"""

import numpy as np

_CACHE = {}


def _build_nc():
    import concourse.bacc as bacc
    import concourse.mybir as mybir
    import concourse.tile as tile

    F32 = mybir.dt.float32
    BF16 = mybir.dt.bfloat16
    MULT = mybir.AluOpType.mult
    ADD = mybir.AluOpType.add
    EQ = mybir.AluOpType.is_equal

    nc = bacc.Bacc("TRN2", target_bir_lowering=False, debug=False, num_devices=8)
    gw = nc.dram_tensor("gw", [49, 518, 518], F32, kind="ExternalInput").ap()
    hn = nc.dram_tensor("hn", [512, 512], F32, kind="ExternalInput").ap()
    h0 = nc.dram_tensor("h0", [512, 512], F32, kind="ExternalInput").ap()
    out = nc.dram_tensor("out", [512, 512], F32, kind="ExternalOutput").ap()

    with tile.TileContext(nc) as tc:
        with (
            tc.tile_pool(name="persist", bufs=1) as pp,
            tc.tile_pool(name="wf", bufs=6) as wfp,
            tc.tile_pool(name="wb", bufs=5) as wbp,
            tc.tile_pool(name="prod", bufs=2) as prp,
            tc.tile_pool(name="wq", bufs=8) as wqp,
            tc.tile_pool(name="wbq", bufs=4) as wbqp,
            tc.tile_pool(name="pq", bufs=4) as pqp,
            tc.tile_pool(name="psum", bufs=2, space="PSUM") as psp,
        ):
            hn_r = hn.rearrange("(b p) x -> p b x", p=128)
            hnf = pp.tile([128, 4, 512], F32)
            nc.sync.dma_start(out=hnf[:], in_=hn_r)
            h0f = pp.tile([128, 4, 512], F32)
            nc.scalar.dma_start(out=h0f[:], in_=h0.rearrange("(b p) x -> p b x", p=128))
            hnb = pp.tile([128, 4, 512], BF16)
            nc.scalar.copy(out=hnb[:], in_=hnf[:])
            h0b = pp.tile([128, 4, 512], BF16)
            nc.scalar.copy(out=h0b[:], in_=h0f[:])

            # Shift matrices for the TensorEngine halo-plane builds.
            # Wm[k, c] = 1 iff c == k+3; the lhsT view Wm[:, 3+d:131+d] is then
            # [k, m] = 1 iff k == m+d, i.e. out[m] = hn_r[m+d] (rows that stay
            # within the partition block; out-of-range rows come out 0, which
            # is exactly the zero padding beyond the image edge).
            # Wc holds the two carry diagonals (c == k+131 for d>0, c == k-125
            # for d<0); the same view indexing turns it into the <=3-row
            # cross-block carry matrix, accumulated from block b+/-1.
            Wm = pp.tile([128, 134], BF16)
            Wc = pp.tile([128, 134], BF16)
            nc.vector.memset(Wm[:], 1.0)
            nc.vector.memset(Wc[:], 1.0)
            nc.gpsimd.affine_select(
                out=Wm[:], in_=Wm[:], pattern=[[1, 134]], compare_op=EQ,
                fill=0.0, base=-3, channel_multiplier=-1,
            )
            nc.gpsimd.affine_select(
                out=Wc[:, 0:3], in_=Wc[:, 0:3], pattern=[[1, 3]], compare_op=EQ,
                fill=0.0, base=125, channel_multiplier=-1,
            )
            nc.gpsimd.affine_select(
                out=Wc[:, 3:134], in_=Wc[:, 3:134], pattern=[[1, 131]], compare_op=EQ,
                fill=0.0, base=-128, channel_multiplier=-1,
            )

            # Halo tensors: s0[p, k, b, u] = hn[128b+p+k-3, u-3] (zero outside
            # the image), s1 the same data one u-slot later so odd-j taps read
            # 4B-aligned.
            s0 = pp.tile([128, 7, 4, 520], BF16, tag="s0")
            s1 = pp.tile([128, 7, 4, 520], BF16, tag="s1")
            nc.vector.memset(s0[:, :, :, 0:3], 0.0)
            nc.vector.memset(s0[:, :, :, 515:520], 0.0)
            nc.vector.memset(s1[:, :, :, 0:4], 0.0)
            nc.vector.memset(s1[:, :, :, 516:520], 0.0)

            def build_plane(k):
                d = k - 3
                if d == 0:
                    nc.vector.tensor_copy(s0[:, 3, :, 3:515], hnb[:])
                    nc.vector.tensor_copy(s1[:, 3, :, 4:516], hnb[:])
                    return
                sgn = 1 if d > 0 else -1
                pt = psp.tile([128, 4, 512], F32, tag="pt")
                for b in range(4):
                    carry = 0 <= b + sgn <= 3
                    nc.tensor.matmul(
                        pt[:, b, :], Wm[:, 3 + d : 131 + d], hnb[:, b, :],
                        start=True, stop=not carry,
                    )
                    if carry:
                        nc.tensor.matmul(
                            pt[:, b, :], Wc[:, 3 + d : 131 + d], hnb[:, b + sgn, :],
                            start=False, stop=True,
                        )
                nc.vector.tensor_copy(s0[:, k, :, 3:515], pt[:])
                nc.vector.tensor_copy(s1[:, k, :, 4:516], pt[:])

            acc = pp.tile([128, 4, 512], BF16)
            outf = pp.tile([128, 4, 512], F32)
            out_ap = out.rearrange("(b p) x -> p b x", p=128)

            def src_for(t):
                i, j = t // 7, t % 7
                if t == 24:
                    return h0b[:]
                if j % 2 == 0:
                    return s0[:, 6 - i, :, 6 - j : 518 - j]
                return s1[:, 6 - i, :, 7 - j : 519 - j]

            # Taps 0..45 stream full-tile, all issued from the Sync ring:
            # that ring carries no compute-gated instruction, so DMA issue
            # never convoys behind a cast's semaphore wait. Plane k=6-i is
            # built one image-row ahead of the taps that read it.
            build_plane(6)
            for t in range(46):
                i, j = t // 7, t % 7
                if j == 0 and i < 6:
                    build_plane(5 - i)
                wf = wfp.tile([128, 4, 512], F32, tag="wf")
                nc.sync.dma_start(
                    out=wf[:],
                    in_=gw[t, 3:515, 3:515].rearrange("(b p) x -> p b x", p=128),
                )
                # bf16 weight cast on the Scalar engine (2x DVE multiply).
                wb = wbp.tile([128, 4, 512], BF16, tag="wb")
                nc.scalar.copy(out=wb[:], in_=wf[:])
                if t == 0:
                    nc.vector.tensor_tensor(
                        out=acc[:], in0=wb[:], in1=src_for(t), op=MULT
                    )
                else:
                    prod = prp.tile([128, 4, 512], BF16, tag="prod")
                    nc.vector.tensor_tensor(
                        out=prod[:], in0=wb[:], in1=src_for(t), op=MULT
                    )
                    nc.vector.tensor_tensor(
                        out=acc[:], in0=acc[:], in1=prod[:], op=ADD
                    )

            # Tail: the last three taps run block-striped (t-major quarter
            # DMAs at the end of the weight stream) so no full-tile
            # cast+multiply+add chain trails the final weight bytes. Blocks
            # 0-2 drain on the DVE, block 3 on GpSimd, so the four
            # accumulate/store chains overlap; the final tap is applied in
            # f32 straight into the output tile (no cast on the drain).
            # Stores split across the Sync/Scalar rings, whose sequencers
            # are idle by then.
            wq = {}
            for t in (46, 47, 48):
                for b in range(4):
                    w = wqp.tile([128, 512], F32, tag="wq")
                    nc.sync.dma_start(
                        out=w[:], in_=gw[t, 3 + 128 * b : 131 + 128 * b, 3:515]
                    )
                    wq[t, b] = w
            wbq = {}
            for t in (46, 47):
                for b in range(4):
                    w = wbqp.tile([128, 512], BF16, tag="wbq")
                    nc.scalar.copy(out=w[:], in_=wq[t, b][:])
                    wbq[t, b] = w

            def tail_chain(eng, blocks):
                for t in (46, 47, 48):
                    st = src_for(t)
                    for b in blocks:
                        if t < 48:
                            prod = pqp.tile([128, 512], BF16, tag="pq")
                            eng.tensor_tensor(
                                out=prod[:], in0=wbq[t, b][:], in1=st[:, b, :],
                                op=MULT,
                            )
                            eng.tensor_tensor(
                                out=acc[:, b, :], in0=acc[:, b, :], in1=prod[:],
                                op=ADD,
                            )
                        else:
                            prodf = pqp.tile([128, 512], F32, tag="pf")
                            eng.tensor_tensor(
                                out=prodf[:], in0=wq[t, b][:], in1=st[:, b, :],
                                op=MULT,
                            )
                            eng.tensor_tensor(
                                out=outf[:, b, :], in0=acc[:, b, :],
                                in1=prodf[:], op=ADD,
                            )

            tail_chain(nc.vector, (0, 1, 2))
            tail_chain(nc.gpsimd, (3,))
            for b, eng in ((0, nc.sync), (1, nc.scalar), (2, nc.sync), (3, nc.scalar)):
                eng.dma_start(out=out_ap[:, b, :], in_=outf[:, b, :])

    nc.compile()
    return nc


def get_nc():
    if "nc" not in _CACHE:
        _CACHE["nc"] = _build_nc()
    return _CACHE["nc"]


def kernel(guide_weight, hn, h0):
    from concourse.bass_utils import run_bass_kernel_spmd

    nc = get_nc()
    in_maps = [
        {
            "gw": np.ascontiguousarray(guide_weight[b], dtype=np.float32),
            "hn": np.ascontiguousarray(hn[b, 0], dtype=np.float32),
            "h0": np.ascontiguousarray(h0[b, 0], dtype=np.float32),
        }
        for b in range(8)
    ]
    res = run_bass_kernel_spmd(nc, in_maps, core_ids=list(range(8)))
    return np.stack([res.results[b]["out"] for b in range(8)])[:, None].astype(
        np.float32
    )
